# revision 19
# baseline (speedup 1.0000x reference)
"""Transformer encoder layer (LN -> MHA -> residual -> LN -> MLP -> residual)
on 8 Trainium2 NeuronCores.

Sharding: token-parallel over the 4096 (batch*seq) tokens, 512 query-tokens
per core; the 4 cores sharing a batch each redundantly compute the full
2048-token K/V for that batch, so no collectives are needed.

v3 design:
  * LayerNorm-1 is applied ON THE HOST (input-only dependent, exact same
    algebra); the kernel receives xhat^T directly in fp8.  The LN affine
    params are folded into the QKV/MLP1 weights as before.
  * The heavy GEMMs (Q/K/V projections, attn@V, MLP1, MLP2) run in
    fp8e4 (e4m3) with MatmulPerfMode.DoubleRow: each matmul contracts
    2x128 rows at ~the cost of one bf16 matmul.  Weights are pre-scaled
    (x32 / x64) on the host so they sit in e4m3's normal range; the
    descale rides existing drain ops.  The ones-column of V is 32.0 so
    softmax normalization cancels the V scale exactly.
  * Scores stay bf16.  Score PSUM tiles are drained to SBUF (bf16) by
    the DVE, and exp() runs in half-headpair batches ([128,8192] per
    ACTIVATE) from SBUF: ScalarE is fully decoupled from the PE's
    score matmuls instead of ping-ponging on a shared PSUM buffer.
  * Softmax denominators ride a 32.0-column of V through the attn@V
    accumulation; the reciprocal row is broadcast across partitions by
    the (otherwise idle) GPSIMD engine, not a PE matmul.
"""

import numpy as np
import ml_dtypes

import concourse.bass as bass
import concourse.mybir as mybir
from concourse import bacc
from concourse.tile import TileContext
from concourse.bass_utils import run_bass_kernel_spmd
from concourse.masks import make_identity

F32 = mybir.dt.float32
BF16 = mybir.dt.bfloat16
F8 = mybir.dt.float8e4
AF = mybir.ActivationFunctionType
ALU = mybir.AluOpType
DR = mybir.MatmulPerfMode.DoubleRow

B, S, D = 2, 2048, 1024
H, HD = 16, 64
DFF = 4 * D
NCORES = 8
QT = 512
EPS = 1e-5
WS = 32.0   # qkv / mlp1 weight pre-scale (host)
WS2 = 64.0  # mlp2 weight pre-scale (host)


def _attention(nc, tc, cpool, attn128, late_dmas):
    """Q/K/V projections + attention; fills attn128 with normalized attn^T."""
    XHT8 = nc.declare_dram_parameter("xht8", [D, S], F8, isOutput=False)
    XQHT8 = nc.declare_dram_parameter("xqht8", [D, QT], F8, isOutput=False)
    WQ8 = nc.declare_dram_parameter("wq8", [D, D], F8, isOutput=False)
    WK8 = nc.declare_dram_parameter("wk8", [D, D], F8, isOutput=False)
    WV8 = nc.declare_dram_parameter("wv8", [D, D], F8, isOutput=False)
    BQ = nc.declare_dram_parameter("bq", [D], F32, isOutput=False)
    BK = nc.declare_dram_parameter("bk", [D], F32, isOutput=False)
    BV32 = nc.declare_dram_parameter("bv32", [D], F32, isOutput=False)

    with (
        tc.tile_pool(name="attp", bufs=1) as attp,
        tc.tile_pool(name="Pp", bufs=2) as Pp,
        tc.tile_pool(name="dsm", bufs=3) as dsm,
    ):
        # tiny DMAs first: they unblock the projection drains
        bqT = cpool.tile([128, 8], F32)
        nc.sync.dma_start(out=bqT, in_=BQ[:].rearrange("(t p) -> p t", p=128))
        bkT = cpool.tile([128, 8], F32)
        nc.sync.dma_start(out=bkT, in_=BK[:].rearrange("(t p) -> p t", p=128))
        bv32_bc = cpool.tile([128, D], F32)
        nc.sync.dma_start(out=bv32_bc, in_=BV32[:].partition_broadcast(128))
        ones64 = cpool.tile([1, 64], BF16)
        nc.vector.memset(ones64, 1.0)

        # critical-path DMAs
        hqT = attp.tile([128, 8, QT], F8)
        nc.sync.dma_start(out=hqT, in_=XQHT8[:].rearrange("(t p) n -> p t n", p=128))
        wq8 = attp.tile([128, 8, D], F8)
        nc.sync.dma_start(out=wq8, in_=WQ8[:].rearrange("(t p) n -> p t n", p=128))
        wk8 = attp.tile([128, 8, D], F8)
        nc.sync.dma_start(out=wk8, in_=WK8[:].rearrange("(t p) n -> p t n", p=128))
        hT = attp.tile([128, 8, S], F8)
        for _nb in range(4):
            nc.sync.dma_start(
                out=hT[:, :, _nb * 512:(_nb + 1) * 512],
                in_=XHT8[:].rearrange("(t p) n -> p t n", p=128)[
                    :, :, _nb * 512:(_nb + 1) * 512
                ],
            )
        wv8 = attp.tile([128, 8, D], F8)
        nc.sync.dma_start(out=wv8, in_=WV8[:].rearrange("(t p) n -> p t n", p=128))
        for out_t, in_ap in late_dmas:
            nc.sync.dma_start(out=out_t, in_=in_ap)

        Q_sb = attp.tile([128, 8, QT], BF16)   # Q^T  [hd(2 heads), ht, q]
        KT = attp.tile([128, 8, S], BF16)      # K^T  [hd(2 heads), ht, keys]
        V = attp.tile([128, 16, 16, 80], F8)   # [key128, st, head, hd+scale+pad]
        nc.vector.memset(V[:, :, :, 64:65], WS)

        # ---- scores -> exp -> attn@V, fully slot-scheduled.
        # P is slot-major: slot s = 2*kt + hp; exp consumes 3-bank score
        # tiles so two of them double-buffer within 6 PSUM banks, leaving
        # 2 banks (tag aux) for Q/K/V projection blocks and the attn@V
        # accumulator.  Only (Q,K) head-tile 0 runs ahead of the loop;
        # the rest feed a fill queue drained inside the exp-paced slots.
        P_tiles = {}

        with (
            tc.tile_pool(name="psS", bufs=2, space="PSUM") as psS,
            tc.tile_pool(name="psX", bufs=2, space="PSUM") as psX,
            tc.tile_pool(name="accp", bufs=3) as accp,
        ):
            def qproj_block(ht):
                psq = psX.tile([128, 512], F32, tag="aux", name=f"psq{ht}")
                for p_ in range(4):
                    nc.tensor.matmul(
                        psq,
                        wq8[:, 2 * p_:2 * p_ + 2, ht * 128:(ht + 1) * 128],
                        hqT[:, 2 * p_:2 * p_ + 2, :],
                        start=(p_ == 0), stop=(p_ == 3), perf_mode=DR,
                    )
                nc.vector.tensor_scalar(
                    Q_sb[:, ht, :], psq, 1.0 / WS, bqT[:, ht:ht + 1],
                    ALU.mult, ALU.add,
                )

            def kproj_block(ht, nb):
                psk = psX.tile([128, 512], F32, tag="aux", name=f"psk{ht}_{nb}")
                for p_ in range(4):
                    nc.tensor.matmul(
                        psk,
                        wk8[:, 2 * p_:2 * p_ + 2, ht * 128:(ht + 1) * 128],
                        hT[:, 2 * p_:2 * p_ + 2, nb * 512:(nb + 1) * 512],
                        start=(p_ == 0), stop=(p_ == 3), perf_mode=DR,
                    )
                nc.vector.tensor_scalar(
                    KT[:, ht, nb * 512:(nb + 1) * 512], psk, 1.0 / WS,
                    bkT[:, ht:ht + 1], ALU.mult, ALU.add,
                )

            def vproj_block(hc, st):
                psv = psX.tile([128, 512], F32, tag="aux", name=f"psv{hc}_{st}")
                for p_ in range(4):
                    nc.tensor.matmul(
                        psv,
                        hT[:, 2 * p_:2 * p_ + 2, st * 128:(st + 1) * 128],
                        wv8[:, 2 * p_:2 * p_ + 2, hc * 512:(hc + 1) * 512],
                        start=(p_ == 0), stop=(p_ == 3), perf_mode=DR,
                    )
                nc.vector.tensor_add(
                    V[:, st, hc * 8:(hc + 1) * 8, 0:64],
                    psv.rearrange("p (h d) -> p h d", h=8),
                    bv32_bc[:, hc * 512:(hc + 1) * 512].rearrange(
                        "p (h d) -> p h d", h=8
                    ),
                )

            # attn@V for one (pair, half) is 8 DoubleRow matmuls, emitted
            # as transient 2-matmul chunks (accumulated into SBUF by the
            # DVE) interleaved between score groups so neither the exp
            # feed nor the PSUM budget is strained.
            acc_live = {}

            def attnv_chunk(jm, hp, c_):
                psc = psX.tile([128, 512], F32, tag="aux", name=f"psc{jm}_{hp}_{c_}")
                Pv = P_tiles[jm].rearrange("p (k h) q -> p h k q", h=2)
                for p_ in (2 * c_, 2 * c_ + 1):
                    nc.tensor.matmul(
                        psc[0:65, :],
                        V[:, 2 * p_:2 * p_ + 2, 2 * jm + hp, 0:65],
                        Pv[:, hp, 2 * p_:2 * p_ + 2, :],
                        start=(p_ == 2 * c_), stop=(p_ == 2 * c_ + 1),
                        perf_mode=DR,
                    )
                if c_ == 0:
                    acc_live[(jm, hp)] = accp.tile(
                        [65, 512], F32, tag="acc", name=f"acc{jm}_{hp}"
                    )
                    nc.vector.tensor_copy(acc_live[(jm, hp)], psc[0:65, :])
                else:
                    acc = acc_live[(jm, hp)]
                    nc.vector.tensor_add(acc, acc, psc[0:65, :])
                if c_ == 3:
                    acc = acc_live[(jm, hp)]
                    dcont = dsm.tile([1, 512], F32, tag="dcont")
                    nc.vector.tensor_copy(dcont, acc[64:65, :])
                    r = dsm.tile([1, 512], F32, tag="r")
                    nc.vector.reciprocal_approx_fast(r, dcont)
                    rbf = dsm.tile([1, 512], BF16, tag="rbf")
                    nc.vector.tensor_copy(rbf, r)
                    rbc = dsm.tile([64, 512], BF16, tag="rbc")
                    nc.gpsimd.partition_broadcast(rbc, rbf)
                    nc.vector.tensor_mul(
                        attn128[64 * hp:64 * hp + 64, jm, :], acc[0:64, :], rbc
                    )

            # prologue: just enough projection for scores of pair 0 (the
            # extra q blocks soak up the wait for the big hT DMA)
            qproj_block(0)
            qproj_block(1)
            qproj_block(2)
            for nb in range(4):
                kproj_block(0, nb)

            # fill queue: (q_n, k_n) due before slot n; V hc0 before the
            # first attn@V chunks (pair 0, slot 1), V hc1 before pair 4.
            fill = []
            fill += [("k", 1, nb) for nb in range(4)]
            fill += [("v", 0, st) for st in range(16)]
            fill += [("k", 2, nb) for nb in range(4)]
            fill += [("q", 3, 0), ("k", 3, 0), ("k", 3, 1), ("k", 3, 2), ("k", 3, 3)]
            fill += [("q", 4, 0)]
            fill += [("v", 1, st) for st in range(16)]
            fill += [("k", 4, nb) for nb in range(4)]
            for n in range(5, 8):
                fill += [("q", n, 0)] + [("k", n, nb) for nb in range(4)]

            def pop_fill(k):
                for _ in range(k):
                    if fill:
                        kind, a, b_ = fill.pop(0)
                        if kind == "q":
                            qproj_block(a)
                        elif kind == "k":
                            kproj_block(a, b_)
                        else:
                            vproj_block(a, b_)

            for j in range(8):
                Pj = Pp.tile([128, 32, 512], F8, tag="P", name=f"P{j}")
                P_tiles[j] = Pj
                for t in range(11):
                    ns = 3 if t < 10 else 2
                    pss = psS.tile([128, 3, 512], F32, tag="pss", name=f"pss{j}_{t}")
                    for i_ in range(ns):
                        s_ = 3 * t + i_
                        kt, hp = s_ // 2, s_ % 2
                        nc.tensor.matmul(
                            pss[:, i_, :],
                            KT[64 * hp:64 * hp + 64, j, kt * 128:(kt + 1) * 128],
                            Q_sb[64 * hp:64 * hp + 64, j, :],
                            start=True, stop=True,
                        )
                    nc.scalar.activation(
                        Pj[:, 3 * t:3 * t + ns, :], pss[:, 0:ns, :],
                        AF.Exp, scale=0.125,
                    )
                    # attn@V chunks for pair j-1: hp0 over t1-t4, hp1 t5-t8
                    if j >= 1 and 1 <= t <= 8:
                        attnv_chunk(j - 1, (t - 1) // 4, (t - 1) % 4)
                    if j == 0:
                        pop_fill(2)
                    elif j <= 2:
                        pop_fill(1)
                    elif t in (0, 1, 2, 8, 9, 10):
                        pop_fill(1)
            for hp in range(2):
                for c_ in range(4):
                    attnv_chunk(7, hp, c_)


def _build():
    nc = bacc.Bacc(None, target_bir_lowering=False)

    XQ32 = nc.declare_dram_parameter("xq32", [QT, D], F32, isOutput=False)
    WO8 = nc.declare_dram_parameter("wo8", [D, D], F8, isOutput=False)
    W18 = nc.declare_dram_parameter("w18", [D, DFF], F8, isOutput=False)
    W28 = nc.declare_dram_parameter("w28", [DFF, D], F8, isOutput=False)
    B1 = nc.declare_dram_parameter("b1", [DFF], F32, isOutput=False)
    B2 = nc.declare_dram_parameter("b2", [D], F32, isOutput=False)
    Y = nc.declare_dram_parameter("y", [QT, D], F32, isOutput=True)

    with TileContext(nc) as tc:
        with (
            tc.tile_pool(name="big", bufs=1) as bigp,
            tc.tile_pool(name="const", bufs=1) as cpool,
        ):
            attn128 = bigp.tile([128, 8, QT], F8)
            b1T = cpool.tile([128, 32], F32)
            nc.sync.dma_start(out=b1T, in_=B1[:].rearrange("(t p) -> p t", p=128))
            eps = cpool.tile([128, 1], F32)
            nc.vector.memset(eps, EPS)

            # tiles used after attention; DMA'd inside _attention (after its
            # critical loads) so they hide under the attention phase
            wo_sb = bigp.tile([128, 8, D], F8)
            xq_sb = bigp.tile([128, 4, D], F32)
            late_dmas = [
                (wo_sb, WO8[:].rearrange("(t p) n -> p t n", p=128)),
                (xq_sb, XQ32[:].rearrange("(t p) n -> p t n", p=128)),
            ]
            _attention(nc, tc, cpool, attn128, late_dmas)

            # ---- out-projection + residual + LN2 + transpose to h2T ----
            with (
                tc.tile_pool(name="x2p", bufs=1) as x2p,
                tc.tile_pool(name="h2p", bufs=1) as h2p,
                tc.tile_pool(name="gp", bufs=1) as gp,
                tc.tile_pool(name="wfp", bufs=3) as wfp,
            ):
                b2_bc = cpool.tile([128, D], F32)
                nc.sync.dma_start(out=b2_bc, in_=B2[:].partition_broadcast(128))
                x2 = x2p.tile([128, 4, D], F32)
                h2T = h2p.tile([128, 8, QT], F8)
                G = gp.tile([128, 32, QT], F8)
                ident = cpool.tile([128, 128], F32)
                make_identity(nc, ident)
                # prefetch the first MLP1 weight chunks under out-proj/LN2
                w1tiles = {}
                for fb in range(2):
                    w1c = wfp.tile([128, 8, 512], F8, tag="w1", name=f"w1c{fb}")
                    nc.sync.dma_start(
                        out=w1c,
                        in_=W18[:, fb * 512:(fb + 1) * 512].rearrange(
                            "(t p) n -> p t n", p=128
                        ),
                    )
                    w1tiles[fb] = w1c
                with (
                    tc.tile_pool(name="lnp2", bufs=2) as lnp2,
                    tc.tile_pool(name="psO", bufs=4, space="PSUM") as psO,
                    tc.tile_pool(name="psT2", bufs=2, space="PSUM") as psT2,
                ):
                    for qt in range(4):
                        po = [
                            psO.tile([128, 512], F32, tag="psO", name=f"po{qt}_{c}")
                            for c in range(2)
                        ]
                        for p_ in range(4):
                            for c in range(2):
                                nc.tensor.matmul(
                                    po[c],
                                    attn128[:, 2 * p_:2 * p_ + 2, qt * 128:(qt + 1) * 128],
                                    wo_sb[:, 2 * p_:2 * p_ + 2, c * 512:(c + 1) * 512],
                                    start=(p_ == 0), stop=(p_ == 3), perf_mode=DR,
                                )
                        # xq_sb already carries x + bo (host-folded)
                        for c in range(2):
                            t1o = lnp2.tile([128, 512], F32, tag="t1o")
                            nc.scalar.mul(t1o, po[c], 1.0 / WS2)
                            nc.vector.tensor_add(
                                x2[:, qt, c * 512:(c + 1) * 512],
                                t1o,
                                xq_sb[:, qt, c * 512:(c + 1) * 512],
                            )
                        xt = x2[:, qt, :]
                        stats = lnp2.tile([128, 2, 6], F32, tag="ln_st")
                        nc.vector.bn_stats(stats[:, 0, :], xt[:, 0:512])
                        nc.vector.bn_stats(stats[:, 1, :], xt[:, 512:1024])
                        mv = lnp2.tile([128, 2], F32, tag="ln_mv")
                        nc.vector.bn_aggr(mv, stats)
                        sd = lnp2.tile([128, 1], F32, tag="ln_sd")
                        nc.scalar.activation(sd, mv[:, 1:2], AF.Sqrt, bias=eps[:, 0:1])
                        rstd = lnp2.tile([128, 1], F32, tag="ln_rs")
                        nc.vector.reciprocal_approx_fast(rstd, sd)
                        hh = lnp2.tile([128, D], F32, tag="ln_h")
                        nc.vector.tensor_scalar(
                            hh, xt, mv[:, 0:1], rstd[:, 0:1], ALU.subtract, ALU.mult
                        )
                        for dt in range(8):
                            pst = psT2.tile([128, 128], F32, tag="tp")
                            nc.tensor.transpose(
                                pst, hh[:, dt * 128:(dt + 1) * 128], ident
                            )
                            nc.vector.tensor_copy(
                                h2T[:, dt, qt * 128:(qt + 1) * 128], pst
                            )

                # ---- MLP (DoubleRow fp8) ----
                with (
                    tc.tile_pool(name="w2p", bufs=8) as w2p,
                    tc.tile_pool(name="psF", bufs=4, space="PSUM") as psF,
                ):
                    w2tiles = {}

                    def w2_fetch(c, fp_):
                        w2t = w2p.tile([128, 2, 512], F8, tag="w2", name=f"w2t{c}_{fp_}")
                        nc.sync.dma_start(
                            out=w2t,
                            in_=W28[:, c * 512:(c + 1) * 512].rearrange(
                                "(t p) n -> p t n", p=128
                            )[:, 2 * fp_:2 * fp_ + 2, :],
                        )
                        w2tiles[(c, fp_)] = w2t

                    for fb in range(8):
                        if fb not in w1tiles:
                            w1c = wfp.tile([128, 8, 512], F8, tag="w1", name=f"w1c{fb}")
                            nc.sync.dma_start(
                                out=w1c,
                                in_=W18[:, fb * 512:(fb + 1) * 512].rearrange(
                                    "(t p) n -> p t n", p=128
                                ),
                            )
                            w1tiles[fb] = w1c
                        w1c = w1tiles[fb]
                        if fb >= 6:  # prefetch first MLP2 weight pairs
                            w2_fetch(0, 2 * (fb - 6))
                            w2_fetch(0, 2 * (fb - 6) + 1)
                        for fo in range(4):
                            ft = fb * 4 + fo
                            psf = psF.tile([128, 512], F32, tag="psF")
                            for p_ in range(4):
                                nc.tensor.matmul(
                                    psf,
                                    w1c[:, 2 * p_:2 * p_ + 2, fo * 128:(fo + 1) * 128],
                                    h2T[:, 2 * p_:2 * p_ + 2, :],
                                    start=(p_ == 0), stop=(p_ == 3), perf_mode=DR,
                                )
                            nc.scalar.activation(
                                G[:, ft, :], psf, AF.Gelu,
                                bias=b1T[:, ft:ft + 1], scale=1.0 / WS,
                            )

                    with (
                        tc.tile_pool(name="yp", bufs=2) as yp,
                        tc.tile_pool(name="psY", bufs=4, space="PSUM") as psY,
                    ):
                        for c in range(2):
                            py = [
                                psY.tile([128, 512], F32, tag="psY", name=f"py{c}_{i}")
                                for i in range(4)
                            ]
                            for fp_ in range(16):
                                if (c, fp_) not in w2tiles:
                                    w2_fetch(c, fp_)
                                w2t = w2tiles[(c, fp_)]
                                if c == 0 and fp_ >= 13:  # prefetch c=1 pairs
                                    w2_fetch(1, fp_ - 13)
                                for qt in range(4):
                                    nc.tensor.matmul(
                                        py[qt],
                                        G[:, 2 * fp_:2 * fp_ + 2, qt * 128:(qt + 1) * 128],
                                        w2t,
                                        start=(fp_ == 0), stop=(fp_ == 15), perf_mode=DR,
                                    )
                            for qt in range(4):
                                t1 = yp.tile([128, 512], F32, tag="yt1")
                                nc.scalar.mul(t1, py[qt], 1.0 / WS2)
                                t2 = yp.tile([128, 512], F32, tag="yt2")
                                nc.vector.tensor_add(
                                    t2, t1, b2_bc[:, c * 512:(c + 1) * 512]
                                )
                                yt = yp.tile([128, 512], F32, tag="yt3")
                                nc.vector.tensor_add(
                                    yt, t2, x2[:, qt, c * 512:(c + 1) * 512]
                                )
                                nc.sync.dma_start(
                                    out=Y[qt * 128:(qt + 1) * 128, c * 512:(c + 1) * 512],
                                    in_=yt,
                                )

    nc.compile()
    return nc


_NC = None


def _get_nc():
    global _NC
    if _NC is None:
        _NC = _build()
    return _NC


def _f8(a):
    return np.ascontiguousarray(
        np.clip(np.asarray(a, dtype=np.float32), -240.0, 240.0).astype(
            ml_dtypes.float8_e4m3
        )
    )


def _make_in_maps(inputs):
    f32 = lambda a: np.ascontiguousarray(np.asarray(a, dtype=np.float32))
    bf16 = lambda a: np.ascontiguousarray(
        np.asarray(a, dtype=np.float32).astype(ml_dtypes.bfloat16)
    )
    x = f32(inputs["x"])
    ln1_g, ln1_b = f32(inputs["ln1_g"]), f32(inputs["ln1_b"])
    ln2_g, ln2_b = f32(inputs["ln2_g"]), f32(inputs["ln2_b"])
    wq, wk, wv, wo = (f32(inputs[k]) for k in ("wq", "wk", "wv", "wo"))
    w1, w2 = f32(inputs["w1"]), f32(inputs["w2"])
    bq, bk, bv, bo = (f32(inputs[k]) for k in ("bq", "bk", "bv", "bo"))
    b1, b2 = f32(inputs["b1"]), f32(inputs["b2"])

    # LayerNorm-1 applied on host (exact algebra; gains folded into weights)
    x64 = x.astype(np.float64)
    mu = x64.mean(axis=2, keepdims=True)
    var = ((x64 - mu) ** 2).mean(axis=2, keepdims=True)
    xhat = ((x64 - mu) / np.sqrt(var + EPS)).astype(np.float32)

    common = {
        "wq8": _f8(WS * ln1_g[:, None] * wq),
        "wk8": _f8(WS * ln1_g[:, None] * wk),
        "wv8": _f8(WS * ln1_g[:, None] * wv),
        "wo8": _f8(WS2 * wo),
        "w18": _f8(WS * ln2_g[:, None] * w1),
        "w28": _f8(WS2 * w2),
        "bq": f32(bq + ln1_b @ wq),
        "bk": f32(bk + ln1_b @ wk),
        "bv32": f32(WS * (bv + ln1_b @ wv)),
        "b1": f32(b1 + ln2_b @ w1),
        "b2": f32(b2),
    }
    in_maps = []
    for c in range(NCORES):
        b = c // 4
        qoff = (c % 4) * QT
        m = dict(common)
        xht = _f8(xhat[b].T)
        m["xht8"] = xht
        m["xqht8"] = np.ascontiguousarray(xht[:, qoff:qoff + QT])
        m["xq32"] = f32(x[b, qoff:qoff + QT] + bo)  # bo folded into residual
        in_maps.append(m)
    return in_maps


def kernel(x, ln1_g, ln1_b, wq, bq, wk, bk, wv, bv, wo, bo, w1, b1, w2, b2, ln2_g, ln2_b):
    inputs = dict(
        x=x, ln1_g=ln1_g, ln1_b=ln1_b, wq=wq, bq=bq, wk=wk, bk=bk, wv=wv, bv=bv,
        wo=wo, bo=bo, w1=w1, b1=b1, w2=w2, b2=b2, ln2_g=ln2_g, ln2_b=ln2_b,
    )
    in_maps = _make_in_maps(inputs)
    nc = _get_nc()
    res = run_bass_kernel_spmd(nc, in_maps, core_ids=list(range(NCORES)))

    y = np.empty((B, S, D), dtype=np.float32)
    for c in range(NCORES):
        b = c // 4
        qoff = (c % 4) * QT
        y[b, qoff:qoff + QT] = res.results[c]["y"]
    return y


# revision 21
# speedup vs baseline: 1.0060x; 1.0060x over previous
"""Transformer encoder layer (LN -> MHA -> residual -> LN -> MLP -> residual)
on 8 Trainium2 NeuronCores.

Sharding: token-parallel over the 4096 (batch*seq) tokens, 512 query-tokens
per core; the 4 cores sharing a batch each redundantly compute the full
2048-token K/V for that batch, so no collectives are needed.

v3 design:
  * LayerNorm-1 is applied ON THE HOST (input-only dependent, exact same
    algebra); the kernel receives xhat^T directly in fp8.  The LN affine
    params are folded into the QKV/MLP1 weights as before.
  * The heavy GEMMs (Q/K/V projections, attn@V, MLP1, MLP2) run in
    fp8e4 (e4m3) with MatmulPerfMode.DoubleRow: each matmul contracts
    2x128 rows at ~the cost of one bf16 matmul.  Weights are pre-scaled
    (x32 / x64) on the host so they sit in e4m3's normal range; the
    descale rides existing drain ops.  The ones-column of V is 32.0 so
    softmax normalization cancels the V scale exactly.
  * Scores stay bf16.  Score PSUM tiles are drained to SBUF (bf16) by
    the DVE, and exp() runs in half-headpair batches ([128,8192] per
    ACTIVATE) from SBUF: ScalarE is fully decoupled from the PE's
    score matmuls instead of ping-ponging on a shared PSUM buffer.
  * Softmax denominators ride a 32.0-column of V through the attn@V
    accumulation; the reciprocal row is broadcast across partitions by
    the (otherwise idle) GPSIMD engine, not a PE matmul.
"""

import numpy as np
import ml_dtypes

import concourse.bass as bass
import concourse.mybir as mybir
from concourse import bacc
from concourse.tile import TileContext
from concourse.bass_utils import run_bass_kernel_spmd
from concourse.masks import make_identity

F32 = mybir.dt.float32
BF16 = mybir.dt.bfloat16
F8 = mybir.dt.float8e4
AF = mybir.ActivationFunctionType
ALU = mybir.AluOpType
DR = mybir.MatmulPerfMode.DoubleRow

B, S, D = 2, 2048, 1024
H, HD = 16, 64
DFF = 4 * D
NCORES = 8
QT = 512
EPS = 1e-5
WS = 32.0   # qkv / mlp1 weight pre-scale (host)
WS2 = 64.0  # mlp2 weight pre-scale (host)


def _attention(nc, tc, cpool, attn128, late_dmas):
    """Q/K/V projections + attention; fills attn128 with normalized attn^T."""
    XHT8 = nc.declare_dram_parameter("xht8", [D, S], F8, isOutput=False)
    XQHT8 = nc.declare_dram_parameter("xqht8", [D, QT], F8, isOutput=False)
    WQ8 = nc.declare_dram_parameter("wq8", [D, D], F8, isOutput=False)
    WK8 = nc.declare_dram_parameter("wk8", [D, D], F8, isOutput=False)
    WV8 = nc.declare_dram_parameter("wv8", [D, D], F8, isOutput=False)
    BQ = nc.declare_dram_parameter("bq", [D], F32, isOutput=False)
    BK = nc.declare_dram_parameter("bk", [D], F32, isOutput=False)
    BV32 = nc.declare_dram_parameter("bv32", [D], F32, isOutput=False)

    with (
        tc.tile_pool(name="attp", bufs=1) as attp,
        tc.tile_pool(name="Pp", bufs=2) as Pp,
        tc.tile_pool(name="dsm", bufs=3) as dsm,
    ):
        # tiny DMAs first: they unblock the projection drains
        bqT = cpool.tile([128, 8], F32)
        nc.sync.dma_start(out=bqT, in_=BQ[:].rearrange("(t p) -> p t", p=128))
        bkT = cpool.tile([128, 8], F32)
        nc.sync.dma_start(out=bkT, in_=BK[:].rearrange("(t p) -> p t", p=128))
        bv32_bc = cpool.tile([128, D], F32)
        nc.sync.dma_start(out=bv32_bc, in_=BV32[:].partition_broadcast(128))
        ones64 = cpool.tile([1, 64], BF16)
        nc.vector.memset(ones64, 1.0)

        # critical-path DMAs
        hqT = attp.tile([128, 8, QT], F8)
        nc.sync.dma_start(out=hqT, in_=XQHT8[:].rearrange("(t p) n -> p t n", p=128))
        wq8 = attp.tile([128, 8, D], F8)
        nc.sync.dma_start(out=wq8, in_=WQ8[:].rearrange("(t p) n -> p t n", p=128))
        wk8 = attp.tile([128, 8, D], F8)
        nc.sync.dma_start(out=wk8, in_=WK8[:].rearrange("(t p) n -> p t n", p=128))
        hT = attp.tile([128, 8, S], F8)
        for _nb in range(4):
            nc.sync.dma_start(
                out=hT[:, :, _nb * 512:(_nb + 1) * 512],
                in_=XHT8[:].rearrange("(t p) n -> p t n", p=128)[
                    :, :, _nb * 512:(_nb + 1) * 512
                ],
            )
        wv8 = attp.tile([128, 8, D], F8)
        nc.sync.dma_start(out=wv8, in_=WV8[:].rearrange("(t p) n -> p t n", p=128))
        for out_t, in_ap in late_dmas:
            nc.sync.dma_start(out=out_t, in_=in_ap)

        Q_sb = attp.tile([128, 8, QT], BF16)   # Q^T  [hd(2 heads), ht, q]
        KT = attp.tile([128, 8, S], BF16)      # K^T  [hd(2 heads), ht, keys]
        V = attp.tile([128, 16, 16, 80], F8)   # [key128, st, head, hd+scale+pad]
        nc.vector.memset(V[:, :, :, 64:65], WS)

        # ---- scores -> exp -> attn@V, fully slot-scheduled.
        # P is slot-major: slot s = 2*kt + hp; exp consumes 3-bank score
        # tiles so two of them double-buffer within 6 PSUM banks, leaving
        # 2 banks (tag aux) for Q/K/V projection blocks and the attn@V
        # accumulator.  Only (Q,K) head-tile 0 runs ahead of the loop;
        # the rest feed a fill queue drained inside the exp-paced slots.
        P_tiles = {}

        with (
            tc.tile_pool(name="psS", bufs=2, space="PSUM") as psS,
            tc.tile_pool(name="psX", bufs=2, space="PSUM") as psX,
            tc.tile_pool(name="accp", bufs=3) as accp,
        ):
            def qproj_block(ht):
                psq = psX.tile([128, 512], F32, tag="aux", name=f"psq{ht}")
                for p_ in range(4):
                    nc.tensor.matmul(
                        psq,
                        wq8[:, 2 * p_:2 * p_ + 2, ht * 128:(ht + 1) * 128],
                        hqT[:, 2 * p_:2 * p_ + 2, :],
                        start=(p_ == 0), stop=(p_ == 3), perf_mode=DR,
                    )
                nc.vector.tensor_scalar(
                    Q_sb[:, ht, :], psq, 1.0 / WS, bqT[:, ht:ht + 1],
                    ALU.mult, ALU.add,
                )

            def kproj_block(ht, nb):
                psk = psX.tile([128, 512], F32, tag="aux", name=f"psk{ht}_{nb}")
                for p_ in range(4):
                    nc.tensor.matmul(
                        psk,
                        wk8[:, 2 * p_:2 * p_ + 2, ht * 128:(ht + 1) * 128],
                        hT[:, 2 * p_:2 * p_ + 2, nb * 512:(nb + 1) * 512],
                        start=(p_ == 0), stop=(p_ == 3), perf_mode=DR,
                    )
                nc.vector.tensor_scalar(
                    KT[:, ht, nb * 512:(nb + 1) * 512], psk, 1.0 / WS,
                    bkT[:, ht:ht + 1], ALU.mult, ALU.add,
                )

            def vproj_block(hc, st):
                psv = psX.tile([128, 512], F32, tag="aux", name=f"psv{hc}_{st}")
                for p_ in range(4):
                    nc.tensor.matmul(
                        psv,
                        hT[:, 2 * p_:2 * p_ + 2, st * 128:(st + 1) * 128],
                        wv8[:, 2 * p_:2 * p_ + 2, hc * 512:(hc + 1) * 512],
                        start=(p_ == 0), stop=(p_ == 3), perf_mode=DR,
                    )
                nc.vector.tensor_add(
                    V[:, st, hc * 8:(hc + 1) * 8, 0:64],
                    psv.rearrange("p (h d) -> p h d", h=8),
                    bv32_bc[:, hc * 512:(hc + 1) * 512].rearrange(
                        "p (h d) -> p h d", h=8
                    ),
                )

            # attn@V for one (pair, half) is 8 DoubleRow matmuls, emitted
            # as transient 2-matmul chunks (accumulated into SBUF by the
            # DVE) interleaved between score groups so neither the exp
            # feed nor the PSUM budget is strained.
            acc_live = {}

            def attnv_chunk(jm, hp, c_):
                psc = psX.tile([128, 512], F32, tag="aux", name=f"psc{jm}_{hp}_{c_}")
                Pv = P_tiles[jm].rearrange("p (k h) q -> p h k q", h=2)
                for p_ in (2 * c_, 2 * c_ + 1):
                    nc.tensor.matmul(
                        psc[0:65, :],
                        V[:, 2 * p_:2 * p_ + 2, 2 * jm + hp, 0:65],
                        Pv[:, hp, 2 * p_:2 * p_ + 2, :],
                        start=(p_ == 2 * c_), stop=(p_ == 2 * c_ + 1),
                        perf_mode=DR,
                    )
                if c_ == 0:
                    acc_live[(jm, hp)] = accp.tile(
                        [65, 512], F32, tag="acc", name=f"acc{jm}_{hp}"
                    )
                    nc.vector.tensor_copy(acc_live[(jm, hp)], psc[0:65, :])
                else:
                    acc = acc_live[(jm, hp)]
                    nc.vector.tensor_add(acc, acc, psc[0:65, :])
                if c_ == 3:
                    acc = acc_live[(jm, hp)]
                    dcont = dsm.tile([1, 512], F32, tag="dcont")
                    nc.vector.tensor_copy(dcont, acc[64:65, :])
                    r = dsm.tile([1, 512], F32, tag="r")
                    nc.vector.reciprocal_approx_fast(r, dcont)
                    rbf = dsm.tile([1, 512], BF16, tag="rbf")
                    nc.vector.tensor_copy(rbf, r)
                    rbc = dsm.tile([64, 512], BF16, tag="rbc")
                    nc.gpsimd.partition_broadcast(rbc, rbf)
                    nc.vector.tensor_mul(
                        attn128[64 * hp:64 * hp + 64, jm, :], acc[0:64, :], rbc
                    )

            # prologue: just enough projection for scores of pair 0 (the
            # extra q blocks soak up the wait for the big hT DMA)
            qproj_block(0)
            qproj_block(1)
            qproj_block(2)
            for nb in range(4):
                kproj_block(0, nb)

            # fill queue: (q_n, k_n) due before slot n; V hc0 before the
            # first attn@V chunks (pair 0, slot 1), V hc1 before pair 4.
            fill = []
            fill += [("k", 1, nb) for nb in range(4)]
            fill += [("v", 0, st) for st in range(16)]
            fill += [("k", 2, nb) for nb in range(4)]
            fill += [("q", 3, 0), ("k", 3, 0), ("k", 3, 1), ("k", 3, 2), ("k", 3, 3)]
            fill += [("q", 4, 0)]
            fill += [("v", 1, st) for st in range(16)]
            fill += [("k", 4, nb) for nb in range(4)]
            for n in range(5, 8):
                fill += [("q", n, 0)] + [("k", n, nb) for nb in range(4)]

            def pop_fill(k):
                for _ in range(k):
                    if fill:
                        kind, a, b_ = fill.pop(0)
                        if kind == "q":
                            qproj_block(a)
                        elif kind == "k":
                            kproj_block(a, b_)
                        else:
                            vproj_block(a, b_)

            for j in range(8):
                Pj = Pp.tile([128, 32, 512], F8, tag="P", name=f"P{j}")
                P_tiles[j] = Pj
                for t in range(11):
                    ns = 3 if t < 10 else 2
                    pss = psS.tile([128, 3, 512], F32, tag="pss", name=f"pss{j}_{t}")
                    for i_ in range(ns):
                        s_ = 3 * t + i_
                        kt, hp = s_ // 2, s_ % 2
                        nc.tensor.matmul(
                            pss[:, i_, :],
                            KT[64 * hp:64 * hp + 64, j, kt * 128:(kt + 1) * 128],
                            Q_sb[64 * hp:64 * hp + 64, j, :],
                            start=True, stop=True,
                        )
                    nc.scalar.activation(
                        Pj[:, 3 * t:3 * t + ns, :], pss[:, 0:ns, :],
                        AF.Exp, scale=0.125,
                    )
                    # attn@V chunks for pair j-1: hp0 over t1-t4, hp1 t5-t8
                    if j >= 1 and 1 <= t <= 8:
                        attnv_chunk(j - 1, (t - 1) // 4, (t - 1) % 4)
                    # pair 7 consumes its own P as soon as the needed exp
                    # batches land, so attention has no serial epilogue
                    if j == 7:
                        if 1 <= t <= 4:
                            attnv_chunk(7, 0, t - 1)
                        elif 6 <= t <= 9:
                            attnv_chunk(7, 1, t - 6)
                    if j == 0:
                        pop_fill(2)
                    elif j <= 2:
                        pop_fill(1)
                    elif t in (0, 1, 2, 8, 9, 10):
                        pop_fill(1)


def _build():
    nc = bacc.Bacc(None, target_bir_lowering=False)

    XQ32 = nc.declare_dram_parameter("xq32", [QT, D], F32, isOutput=False)
    WO8 = nc.declare_dram_parameter("wo8", [D, D], F8, isOutput=False)
    W18 = nc.declare_dram_parameter("w18", [D, DFF], F8, isOutput=False)
    W28 = nc.declare_dram_parameter("w28", [DFF, D], F8, isOutput=False)
    B1 = nc.declare_dram_parameter("b1", [DFF], F32, isOutput=False)
    B2 = nc.declare_dram_parameter("b2", [D], F32, isOutput=False)
    Y = nc.declare_dram_parameter("y", [QT, D], F32, isOutput=True)

    with TileContext(nc) as tc:
        with (
            tc.tile_pool(name="big", bufs=1) as bigp,
            tc.tile_pool(name="const", bufs=1) as cpool,
        ):
            attn128 = bigp.tile([128, 8, QT], F8)
            b1T = cpool.tile([128, 32], F32)
            nc.sync.dma_start(out=b1T, in_=B1[:].rearrange("(t p) -> p t", p=128))
            eps = cpool.tile([128, 1], F32)
            nc.vector.memset(eps, EPS)

            # tiles used after attention; DMA'd inside _attention (after its
            # critical loads) so they hide under the attention phase
            wo_sb = bigp.tile([128, 8, D], F8)
            xq_sb = bigp.tile([128, 4, D], F32)
            late_dmas = [
                (wo_sb, WO8[:].rearrange("(t p) n -> p t n", p=128)),
                (xq_sb, XQ32[:].rearrange("(t p) n -> p t n", p=128)),
            ]
            _attention(nc, tc, cpool, attn128, late_dmas)

            # ---- out-projection + residual + LN2 + transpose to h2T ----
            with (
                tc.tile_pool(name="x2p", bufs=1) as x2p,
                tc.tile_pool(name="h2p", bufs=1) as h2p,
                tc.tile_pool(name="gp", bufs=1) as gp,
                tc.tile_pool(name="wfp", bufs=3) as wfp,
            ):
                b2_bc = cpool.tile([128, D], F32)
                nc.sync.dma_start(out=b2_bc, in_=B2[:].partition_broadcast(128))
                x2 = x2p.tile([128, 4, D], F32)
                h2T = h2p.tile([128, 8, QT], F8)
                G = gp.tile([128, 32, QT], F8)
                ident = cpool.tile([128, 128], F32)
                make_identity(nc, ident)
                # prefetch the first MLP1 weight chunks under out-proj/LN2
                w1tiles = {}
                for fb in range(2):
                    w1c = wfp.tile([128, 8, 512], F8, tag="w1", name=f"w1c{fb}")
                    nc.sync.dma_start(
                        out=w1c,
                        in_=W18[:, fb * 512:(fb + 1) * 512].rearrange(
                            "(t p) n -> p t n", p=128
                        ),
                    )
                    w1tiles[fb] = w1c
                with (
                    tc.tile_pool(name="lnp2", bufs=2) as lnp2,
                    tc.tile_pool(name="psO", bufs=4, space="PSUM") as psO,
                    tc.tile_pool(name="psT2", bufs=2, space="PSUM") as psT2,
                ):
                    for qt in range(4):
                        po = [
                            psO.tile([128, 512], F32, tag="psO", name=f"po{qt}_{c}")
                            for c in range(2)
                        ]
                        for p_ in range(4):
                            for c in range(2):
                                nc.tensor.matmul(
                                    po[c],
                                    attn128[:, 2 * p_:2 * p_ + 2, qt * 128:(qt + 1) * 128],
                                    wo_sb[:, 2 * p_:2 * p_ + 2, c * 512:(c + 1) * 512],
                                    start=(p_ == 0), stop=(p_ == 3), perf_mode=DR,
                                )
                        # xq_sb already carries x + bo (host-folded)
                        for c in range(2):
                            t1o = lnp2.tile([128, 512], F32, tag="t1o")
                            nc.scalar.mul(t1o, po[c], 1.0 / WS2)
                            nc.vector.tensor_add(
                                x2[:, qt, c * 512:(c + 1) * 512],
                                t1o,
                                xq_sb[:, qt, c * 512:(c + 1) * 512],
                            )
                        xt = x2[:, qt, :]
                        stats = lnp2.tile([128, 2, 6], F32, tag="ln_st")
                        nc.vector.bn_stats(stats[:, 0, :], xt[:, 0:512])
                        nc.vector.bn_stats(stats[:, 1, :], xt[:, 512:1024])
                        mv = lnp2.tile([128, 2], F32, tag="ln_mv")
                        nc.vector.bn_aggr(mv, stats)
                        sd = lnp2.tile([128, 1], F32, tag="ln_sd")
                        nc.scalar.activation(sd, mv[:, 1:2], AF.Sqrt, bias=eps[:, 0:1])
                        rstd = lnp2.tile([128, 1], F32, tag="ln_rs")
                        nc.vector.reciprocal_approx_fast(rstd, sd)
                        hh = lnp2.tile([128, D], F32, tag="ln_h")
                        nc.vector.tensor_scalar(
                            hh, xt, mv[:, 0:1], rstd[:, 0:1], ALU.subtract, ALU.mult
                        )
                        for dt in range(8):
                            pst = psT2.tile([128, 128], F32, tag="tp")
                            nc.tensor.transpose(
                                pst, hh[:, dt * 128:(dt + 1) * 128], ident
                            )
                            nc.vector.tensor_copy(
                                h2T[:, dt, qt * 128:(qt + 1) * 128], pst
                            )

                # ---- MLP (DoubleRow fp8) ----
                with (
                    tc.tile_pool(name="w2p", bufs=8) as w2p,
                    tc.tile_pool(name="psF", bufs=4, space="PSUM") as psF,
                ):
                    w2tiles = {}

                    def w2_fetch(c, fp_):
                        w2t = w2p.tile([128, 2, 512], F8, tag="w2", name=f"w2t{c}_{fp_}")
                        nc.sync.dma_start(
                            out=w2t,
                            in_=W28[:, c * 512:(c + 1) * 512].rearrange(
                                "(t p) n -> p t n", p=128
                            )[:, 2 * fp_:2 * fp_ + 2, :],
                        )
                        w2tiles[(c, fp_)] = w2t

                    for fb in range(8):
                        if fb not in w1tiles:
                            w1c = wfp.tile([128, 8, 512], F8, tag="w1", name=f"w1c{fb}")
                            nc.sync.dma_start(
                                out=w1c,
                                in_=W18[:, fb * 512:(fb + 1) * 512].rearrange(
                                    "(t p) n -> p t n", p=128
                                ),
                            )
                            w1tiles[fb] = w1c
                        w1c = w1tiles[fb]
                        if fb >= 6:  # prefetch first MLP2 weight pairs
                            w2_fetch(0, 2 * (fb - 6))
                            w2_fetch(0, 2 * (fb - 6) + 1)
                        for fo in range(4):
                            ft = fb * 4 + fo
                            psf = psF.tile([128, 512], F32, tag="psF")
                            for p_ in range(4):
                                nc.tensor.matmul(
                                    psf,
                                    w1c[:, 2 * p_:2 * p_ + 2, fo * 128:(fo + 1) * 128],
                                    h2T[:, 2 * p_:2 * p_ + 2, :],
                                    start=(p_ == 0), stop=(p_ == 3), perf_mode=DR,
                                )
                            nc.scalar.activation(
                                G[:, ft, :], psf, AF.Gelu,
                                bias=b1T[:, ft:ft + 1], scale=1.0 / WS,
                            )

                    with (
                        tc.tile_pool(name="yp", bufs=2) as yp,
                        tc.tile_pool(name="psY", bufs=4, space="PSUM") as psY,
                    ):
                        for c in range(2):
                            py = [
                                psY.tile([128, 512], F32, tag="psY", name=f"py{c}_{i}")
                                for i in range(4)
                            ]
                            for fp_ in range(16):
                                if (c, fp_) not in w2tiles:
                                    w2_fetch(c, fp_)
                                w2t = w2tiles[(c, fp_)]
                                if c == 0 and fp_ >= 11:  # prefetch c=1 pairs
                                    w2_fetch(1, fp_ - 11)
                                for qt in range(4):
                                    nc.tensor.matmul(
                                        py[qt],
                                        G[:, 2 * fp_:2 * fp_ + 2, qt * 128:(qt + 1) * 128],
                                        w2t,
                                        start=(fp_ == 0), stop=(fp_ == 15), perf_mode=DR,
                                    )
                            for qt in range(4):
                                t1 = yp.tile([128, 512], F32, tag="yt1")
                                nc.scalar.mul(t1, py[qt], 1.0 / WS2)
                                t2 = yp.tile([128, 512], F32, tag="yt2")
                                nc.vector.tensor_add(
                                    t2, t1, b2_bc[:, c * 512:(c + 1) * 512]
                                )
                                yt = yp.tile([128, 512], F32, tag="yt3")
                                nc.vector.tensor_add(
                                    yt, t2, x2[:, qt, c * 512:(c + 1) * 512]
                                )
                                nc.sync.dma_start(
                                    out=Y[qt * 128:(qt + 1) * 128, c * 512:(c + 1) * 512],
                                    in_=yt,
                                )

    nc.compile()
    return nc


_NC = None


def _get_nc():
    global _NC
    if _NC is None:
        _NC = _build()
    return _NC


def _f8(a):
    return np.ascontiguousarray(
        np.clip(np.asarray(a, dtype=np.float32), -240.0, 240.0).astype(
            ml_dtypes.float8_e4m3
        )
    )


def _make_in_maps(inputs):
    f32 = lambda a: np.ascontiguousarray(np.asarray(a, dtype=np.float32))
    bf16 = lambda a: np.ascontiguousarray(
        np.asarray(a, dtype=np.float32).astype(ml_dtypes.bfloat16)
    )
    x = f32(inputs["x"])
    ln1_g, ln1_b = f32(inputs["ln1_g"]), f32(inputs["ln1_b"])
    ln2_g, ln2_b = f32(inputs["ln2_g"]), f32(inputs["ln2_b"])
    wq, wk, wv, wo = (f32(inputs[k]) for k in ("wq", "wk", "wv", "wo"))
    w1, w2 = f32(inputs["w1"]), f32(inputs["w2"])
    bq, bk, bv, bo = (f32(inputs[k]) for k in ("bq", "bk", "bv", "bo"))
    b1, b2 = f32(inputs["b1"]), f32(inputs["b2"])

    # LayerNorm-1 applied on host (exact algebra; gains folded into weights)
    x64 = x.astype(np.float64)
    mu = x64.mean(axis=2, keepdims=True)
    var = ((x64 - mu) ** 2).mean(axis=2, keepdims=True)
    xhat = ((x64 - mu) / np.sqrt(var + EPS)).astype(np.float32)

    common = {
        "wq8": _f8(WS * ln1_g[:, None] * wq),
        "wk8": _f8(WS * ln1_g[:, None] * wk),
        "wv8": _f8(WS * ln1_g[:, None] * wv),
        "wo8": _f8(WS2 * wo),
        "w18": _f8(WS * ln2_g[:, None] * w1),
        "w28": _f8(WS2 * w2),
        "bq": f32(bq + ln1_b @ wq),
        "bk": f32(bk + ln1_b @ wk),
        "bv32": f32(WS * (bv + ln1_b @ wv)),
        "b1": f32(b1 + ln2_b @ w1),
        "b2": f32(b2),
    }
    in_maps = []
    for c in range(NCORES):
        b = c // 4
        qoff = (c % 4) * QT
        m = dict(common)
        xht = _f8(xhat[b].T)
        m["xht8"] = xht
        m["xqht8"] = np.ascontiguousarray(xht[:, qoff:qoff + QT])
        m["xq32"] = f32(x[b, qoff:qoff + QT] + bo)  # bo folded into residual
        in_maps.append(m)
    return in_maps


def kernel(x, ln1_g, ln1_b, wq, bq, wk, bk, wv, bv, wo, bo, w1, b1, w2, b2, ln2_g, ln2_b):
    inputs = dict(
        x=x, ln1_g=ln1_g, ln1_b=ln1_b, wq=wq, bq=bq, wk=wk, bk=bk, wv=wv, bv=bv,
        wo=wo, bo=bo, w1=w1, b1=b1, w2=w2, b2=b2, ln2_g=ln2_g, ln2_b=ln2_b,
    )
    in_maps = _make_in_maps(inputs)
    nc = _get_nc()
    res = run_bass_kernel_spmd(nc, in_maps, core_ids=list(range(NCORES)))

    y = np.empty((B, S, D), dtype=np.float32)
    for c in range(NCORES):
        b = c // 4
        qoff = (c % 4) * QT
        y[b, qoff:qoff + QT] = res.results[c]["y"]
    return y
